# revision 8
# baseline (speedup 1.0000x reference)
"""Trainium2 Bass kernel for nn_ByteShiftPowerOf2 (v3).

Reference semantics per token (B*S tokens, D=128 features):
  val_lo = argmax(x[16:32]); val_hi = argmax(x[32:48]); value = val_lo + 16*val_hi
  shift  = argmax(x[48:64])
  mark = x[0] >= 0.5; shl = x[1] > 0.5; shr = x[2] > 0.5; active = mark & (shl|shr)
  result = shl ? (value << shift) & 255 : value >> shift
  out = x; if active: out[64 + (result & 15)] += 2.0; out[80 + (result >> 4)] += 2.0

Only features 0..2, 16..95 are read and only 64..95 are written, so the
host packs the input to 84 f32 columns per token (336B) and the device
returns just the 32-column band as bf16 (64B); the +2 adds happen in f32
and only the final sum is rounded to bf16 (rel err <= 2^-9). Full output
is assembled on host. HBM traffic per core: 13.1 MB vs 33.6 MB for full
rows.

All-float device pipeline (int32 DVE ops measure ~20x slower than f32;
mod/divide don't exist on TRN2):
  r3 = reduce_max per 16-bin group                          [DVE f32]
  d  = x48 - r3 (== +0 only at max)                         [GPSIMD]
  eq = Relu(d*1e30 + 1)  exact one-hot                      [ACT]
  u  = eq * w, w = [15-s | 240-16s | 2^(15-s)] (descending
       => first-occurrence tie-break like jnp.argmax)       [DVE bf16 2x]
  rw = reduce_max(u) = (15-lo, 240-16hi, 2^(15-shift))      [DVE]
  value = 255 - (rw0+rw1); p = rw2 * 2^-15 = 2^-shift
  2^shift via bf16 bit trick: bits(2^sh) = 32512 - bits(2^-sh)  [ACT on i16]
  q = value * (shl ? 2^sh : 2^-sh)   (bf16-exact: value has <=8
      significand bits, scales are powers of two)
  floor() via round-nearest f32->i32 convert with bias -(0.5 - grid/2);
  all inputs sit on known power-of-2 grids so the RNE convert is exact:
    m = floor(q/256); t = q - 256m; res = floor(t); hi = floor(t/16);
    lo = res - 16*hi
  scatter +2.0 via gpsimd local_scatter (negative idx -> skipped)
  band = x[52:84] + plane  -> bf16                          [GPSIMD]

8 uniform tiles of K=32 tokens/partition pipeline the DMA; the per-token
scalar chain runs once per half-core group of 128 tokens to amortize the
~0.2-0.3us/instruction fixed cost.
"""

import numpy as np
from contextlib import ExitStack

import concourse.bass as bass
import concourse.tile as tile
from concourse import bacc, mybir
from concourse.bass_utils import run_bass_kernel_spmd

B, S, D = 32, 8192, 128
N_CORES = 8
TOK = B * S                       # 262144 tokens
TOK_CORE = TOK // N_CORES         # 32768 tokens per core
P = 128                           # partitions
K = 32                            # tokens per partition per tile
NT = 8                            # tiles per core
GT = 4                            # tiles per group
NG = NT // GT                     # groups (small-op stage instances)
M = K * GT                        # tokens per partition per group (128)
assert P * K * NT == TOK_CORE
C = 84                            # packed input columns per token

F32 = mybir.dt.float32
BF16 = mybir.dt.bfloat16
FP16 = mybir.dt.float16
I32 = mybir.dt.int32
I16 = mybir.dt.int16
Op = mybir.AluOpType
Act = mybir.ActivationFunctionType

OFF8 = -(0.5 - 2.0 ** -9)         # floor bias, fraction grid 2^-8
OFF15 = -(0.5 - 2.0 ** -16)       # floor bias, fraction grid 2^-15
OFF19 = -(0.5 - 2.0 ** -20)       # floor bias, fraction grid 2^-19


def _build():
    nc = bacc.Bacc("TRN2", debug=False, enable_asserts=False, num_devices=N_CORES)
    x = nc.dram_tensor("x", [TOK_CORE, C], F32, kind="ExternalInput").ap()
    y = nc.dram_tensor("y", [TOK_CORE, 32], BF16, kind="ExternalOutput").ap()

    with tile.TileContext(nc) as tc, ExitStack() as ctx:
        io_pool = ctx.enter_context(tc.tile_pool(name="io", bufs=7))
        eq_pool = ctx.enter_context(tc.tile_pool(name="eq", bufs=3))
        r3_pool = ctx.enter_context(tc.tile_pool(name="r3", bufs=3))
        pl_pool = ctx.enter_context(tc.tile_pool(name="pl", bufs=3))
        bd_pool = ctx.enter_context(tc.tile_pool(name="bd", bufs=3))
        gr_pool = ctx.enter_context(tc.tile_pool(name="gr", bufs=2))
        const_pool = ctx.enter_context(tc.tile_pool(name="const", bufs=1))

        # ---- in-DMAs first on the Sync queue: nothing else runs on Sync,
        # so loads start at t~0 and overlap all constant setup below.
        xts = []
        for t in range(NT):
            xt = io_pool.tile([P, K * C], F32, tag="xt")
            x_t = x[t * P * K:(t + 1) * P * K].rearrange("(p j) f -> p (j f)", p=P)
            nc.sync.dma_start(xt[:], x_t)
            xts.append(xt)

        # ---- constants; local_scatter warmup first (~6us Q7 IRAM load) ----
        data2 = const_pool.tile([P, K * 2], BF16)            # scatter payload
        nc.gpsimd.memset(data2[:], 2.0)
        wu_idx = const_pool.tile([P, 2], I16)
        nc.gpsimd.memset(wu_idx[:], -1)
        wu_dst = const_pool.tile([P, 4], BF16)
        nc.gpsimd.local_scatter(wu_dst[:], data2[:, 0:2], wu_idx[:],
                                channels=P, num_elems=4, num_idxs=2)

        # w48: [15-s | 240-16s | 2^(15-s)] per 16-bin group (all descending)
        w48 = const_pool.tile([P, 48], BF16)
        tmp_i = const_pool.tile([P, 32], I32)
        nc.gpsimd.iota(tmp_i[:], pattern=[[0, 2], [-1, 16]], base=15,
                       channel_multiplier=0)
        nc.scalar.copy(w48[:, 0:32], tmp_i[:])
        nc.vector.tensor_scalar(w48[:, 16:32], w48[:, 16:32], 16.0, None,
                                op0=Op.mult)
        tmp_h = const_pool.tile([P, 16], I16)               # bf16 bits of 2^(15-s)
        nc.gpsimd.iota(tmp_h[:], pattern=[[-128, 16]], base=(127 + 15) << 7,
                       channel_multiplier=0)
        nc.scalar.copy(w48[:, 32:48], tmp_h[:].bitcast(BF16))
        wrep = const_pool.tile([P, K * 48], BF16)           # repeat per token
        nc.scalar.copy(wrep[:].rearrange("p (j f) -> p j f", j=K),
                       w48[:].unsqueeze(1).broadcast_to([P, K, 48]))

        # jvec: per-token scatter base (j*32, j*32+16), j = token mod K
        jveci = const_pool.tile([P, M, 2], I32)
        nc.gpsimd.iota(jveci[:], pattern=[[0, GT], [32, K], [16, 2]], base=0,
                       channel_multiplier=0)
        jvecf = const_pool.tile([P, M, 2], FP16)
        nc.scalar.copy(jvecf[:], jveci[:])

        c8192 = const_pool.tile([P, 1], F32)
        nc.gpsimd.memset(c8192[:], 8192.0)

        # whole-core staging for the small-op stage
        rw_st = const_pool.tile([P, NT * K, 3], BF16)
        fl_st = const_pool.tile([P, NT * K, 3], BF16)

        # ---- per-tile heavy passes ----
        for t in range(NT):
            xt = xts[t]
            x3 = xt[:].rearrange("p (j f) -> p j f", j=K)
            x48 = x3[:, :, 4:52].rearrange("p j (g s) -> p j g s", s=16)

            r3 = r3_pool.tile([P, K, 3], F32, tag="r3")
            nc.vector.tensor_reduce(r3[:], x48, axis=mybir.AxisListType.X,
                                    op=Op.max)
            equ = eq_pool.tile([P, K * 48], BF16, tag="equ")
            eq4 = equ[:].rearrange("p (j g s) -> p j g s", j=K, g=3)
            r3b = r3[:].unsqueeze(3).broadcast_to([P, K, 3, 16])
            nc.gpsimd.tensor_tensor(eq4, x48, r3b, op=Op.subtract)
            nc.scalar.activation(equ[:], equ[:], Act.Relu, bias=1.0, scale=1e30)
            nc.vector.tensor_tensor(equ[:], equ[:], wrep[:], op=Op.mult)
            nc.vector.tensor_reduce(rw_st[:, t * K:(t + 1) * K, :], eq4,
                                    axis=mybir.AxisListType.X, op=Op.max)
            nc.vector.tensor_scalar(fl_st[:, t * K:(t + 1) * K, :],
                                    x3[:, :, 0:3], 0.5, None, op0=Op.is_gt)

        # ---- per-group small-op stage + per-tile scatter/band/out ----
        for g in range(NG):
            o = g * M
            rw = rw_st[:, o:o + M, :]
            fl = fl_st[:, o:o + M, :]

            t01 = gr_pool.tile([P, M], BF16, tag="t01")
            nc.vector.tensor_tensor(t01[:], rw[:, :, 0], rw[:, :, 1], op=Op.add)
            value = gr_pool.tile([P, M], BF16, tag="value")
            nc.scalar.activation(value[:], t01[:], Act.Copy, bias=255.0,
                                 scale=-1.0)
            p = gr_pool.tile([P, M], BF16, tag="p")         # 2^-shift
            nc.scalar.activation(p[:], rw[:, :, 2], Act.Copy, scale=2.0 ** -15)
            # 2^shift from bf16 bit identity: bits(2^sh) = 32512 - bits(2^-sh)
            pf = gr_pool.tile([P, M], I16, tag="pf")
            nc.scalar.activation(pf[:], p[:].bitcast(I16), Act.Copy,
                                 bias=32512.0, scale=-1.0)
            s12 = gr_pool.tile([P, M], BF16, tag="s12")
            nc.gpsimd.tensor_tensor(s12[:], fl[:, :, 1], fl[:, :, 2], op=Op.add)
            a = gr_pool.tile([P, M], BF16, tag="a")
            nc.gpsimd.tensor_tensor(a[:], fl[:, :, 0], s12[:], op=Op.mult)
            offp = gr_pool.tile([P, M, 2], FP16, tag="offp")
            nc.scalar.activation(offp[:, :, 0], a[:], Act.Relu, bias=c8192[:],
                                 scale=-8192.0)
            nc.scalar.activation(offp[:, :, 1], a[:], Act.Relu, bias=c8192[:],
                                 scale=-8192.0)
            # p := shl ? 2^shift : 2^-shift   (shl wins over shr, as in ref)
            shli = gr_pool.tile([P, M], I16, tag="shli")
            nc.scalar.copy(shli[:], fl[:, :, 1])
            nc.vector.copy_predicated(p[:], shli[:], pf[:].bitcast(BF16))
            q = gr_pool.tile([P, M], BF16, tag="q")
            nc.vector.tensor_tensor(q[:], value[:], p[:], op=Op.mult)
            # t = q mod 256 via m = floor(q/256)
            m_i = gr_pool.tile([P, M], I32, tag="m_i")
            nc.scalar.activation(m_i[:], q[:], Act.Copy, bias=OFF8,
                                 scale=1.0 / 256.0)
            m_sc = gr_pool.tile([P, M], BF16, tag="m_sc")   # -256*m
            nc.scalar.activation(m_sc[:], m_i[:], Act.Copy, scale=-256.0)
            tq = gr_pool.tile([P, M], BF16, tag="tq")
            nc.vector.tensor_tensor(tq[:], q[:], m_sc[:], op=Op.add)
            # res = floor(t); hi = floor(t/16); lo = res - 16*hi
            res_i = gr_pool.tile([P, M], I32, tag="res_i")
            nc.scalar.activation(res_i[:], tq[:], Act.Copy, bias=OFF15)
            hi_i = gr_pool.tile([P, M], I32, tag="hi_i")
            nc.scalar.activation(hi_i[:], tq[:], Act.Copy, bias=OFF19,
                                 scale=1.0 / 16.0)
            res_f = gr_pool.tile([P, M], BF16, tag="res_f")
            nc.scalar.copy(res_f[:], res_i[:])
            hi16 = gr_pool.tile([P, M], BF16, tag="hi16")   # 16*hi
            nc.scalar.activation(hi16[:], hi_i[:], Act.Copy, scale=16.0)
            pair = gr_pool.tile([P, M, 2], FP16, tag="pair")
            nc.scalar.copy(pair[:, :, 1], hi_i[:])
            nc.vector.tensor_tensor(pair[:, :, 0], res_f[:], hi16[:],
                                    op=Op.subtract)
            jmo = gr_pool.tile([P, M, 2], FP16, tag="jmo")
            nc.gpsimd.tensor_tensor(jmo[:], jvecf[:], offp[:], op=Op.subtract)
            idxf = gr_pool.tile([P, M, 2], FP16, tag="idxf")
            nc.vector.tensor_tensor(idxf[:], pair[:], jmo[:], op=Op.add)
            idx16 = gr_pool.tile([P, M * 2], I16, tag="idx16")
            nc.scalar.copy(idx16[:], idxf[:].rearrange("p j l -> p (j l)"))

            for tt in range(GT):
                t = g * GT + tt
                xt = xts[t]
                x3 = xt[:].rearrange("p (j f) -> p j f", j=K)
                plane = pl_pool.tile([P, K * 32], BF16, tag="plane")
                nc.gpsimd.local_scatter(
                    plane[:], data2[:], idx16[:, tt * 2 * K:(tt + 1) * 2 * K],
                    channels=P, num_elems=K * 32, num_idxs=K * 2)
                band = bd_pool.tile([P, K * 32], BF16, tag="band")
                nc.gpsimd.tensor_tensor(
                    band[:].rearrange("p (j s) -> p j s", j=K),
                    x3[:, :, 52:84],
                    plane[:].rearrange("p (j s) -> p j s", j=K),
                    op=Op.add)
                y_t = y[t * P * K:(t + 1) * P * K].rearrange(
                    "(p j) f -> p (j f)", p=P)
                nc.scalar.dma_start(y_t, band[:])

    nc.compile()
    return nc


_NC_CACHE = None


def _get_nc():
    global _NC_CACHE
    if _NC_CACHE is None:
        _NC_CACHE = _build()
    return _NC_CACHE


def kernel(x_bd: np.ndarray, _trace: bool = False, **_kw):
    assert x_bd.shape == (B, S, D) and x_bd.dtype == np.float32
    nc = _get_nc()
    xf = np.ascontiguousarray(x_bd).reshape(TOK, D)
    x84 = np.empty((TOK, C), np.float32)
    x84[:, 0:3] = xf[:, 0:3]
    x84[:, 3] = 0.0
    x84[:, 4:52] = xf[:, 16:64]
    x84[:, 52:84] = xf[:, 64:96]
    in_maps = [{"x": x84[c * TOK_CORE:(c + 1) * TOK_CORE]} for c in range(N_CORES)]
    res = run_bass_kernel_spmd(nc, in_maps, core_ids=list(range(N_CORES)),
                               trace=_trace)
    band = np.concatenate([np.asarray(res.results[c]["y"])
                           for c in range(N_CORES)], axis=0)
    out = np.array(xf, copy=True)
    out[:, 64:96] = band.astype(np.float32)
    out = out.reshape(B, S, D)
    if _trace:
        return out, res
    return out


# revision 9
# speedup vs baseline: 1.1007x; 1.1007x over previous
"""Trainium2 Bass kernel for nn_ByteShiftPowerOf2 (v4).

Reference semantics per token (B*S tokens, D=128 features):
  val_lo = argmax(x[16:32]); val_hi = argmax(x[32:48]); value = val_lo + 16*val_hi
  shift  = argmax(x[48:64])
  mark = x[0] >= 0.5; shl = x[1] > 0.5; shr = x[2] > 0.5; active = mark & (shl|shr)
  result = shl ? (value << shift) & 255 : value >> shift
  out = x; if active: out[64 + (result & 15)] += 2.0; out[80 + (result >> 4)] += 2.0

I/O diet: only features 16..95 feed device compute and only 64..95 change.
The host packs [48 alu cols | 32 band cols] f32 (320B/token) plus a 4B/token
side tensor flg = (jmo0 fp16, shl i16), where jmo0 = 32*j - 8192*(1-active)
is the per-token scatter base minus deactivation offset (host-computable:
it doesn't depend on the device argmax). The device returns just the 32-col
band as bf16; the +2 adds happen in f32 and only the final sum is rounded
to bf16 (rel err <= 2^-9). Full output is assembled host-side.
HBM/core: 11.5 MB in + 2.1 MB out vs 33.6 MB for full rows.

All-float device pipeline (int32 DVE ops measure ~20x slower than f32;
mod/divide don't exist on TRN2):
  r3 = reduce_max per 16-bin group                         [DVE f32]
  eq: d = x48 - r3 then Relu(d*1e30 + 1)  exact one-hot    [GPSIMD + ACT]
      (tiles 0..: V is_ge variant used for engine balance)
  u  = eq * w, w = [15-s | 240-16s | 2^(15-s)] (descending
       => first-occurrence tie-break like jnp.argmax)      [DVE bf16 2x]
  rw = reduce_max(u) = (15-lo, 240-16hi, 2^(15-shift))     [DVE]
  value = 255 - (rw0+rw1); p = rw2 * 2^-15 = 2^-shift
  2^shift via bf16 bit identity bits(2^sh) = 32512 - bits(2^-sh)  [ACT i16]
  q = value * (shl ? 2^sh : 2^-sh)  (bf16-exact: value has <=8
      significand bits, scales are powers of two)
  floor() via round-nearest f32->i32 convert biased by -(0.5 - grid/2);
  all inputs sit on known power-of-2 grids so the RNE convert is exact:
    m = floor(q/256); t = q - 256m; res = floor(t); hi = floor(t/16);
    lo = res - 16*hi
  idx = (lo, hi+16) + jmo0; scatter +2.0 via local_scatter (neg -> skip)
  band = x_band + plane -> bf16                            [GPSIMD/DVE]

6 tiles (K tokens/partition: 30,45,45,46,45,45) pipeline the DMA; the
per-token scalar chain runs once per group of tiles (sizes 120/91/45) to
amortize per-instruction fixed costs, with groups shrinking toward the
end for a short drain tail. Emission is interleaved per group so every
engine's in-order queue follows pipeline order.
"""

import numpy as np
from contextlib import ExitStack

import concourse.bass as bass
import concourse.tile as tile
from concourse import bacc, mybir
from concourse.bass_utils import run_bass_kernel_spmd

B, S, D = 32, 8192, 128
N_CORES = 8
TOK = B * S                       # 262144 tokens
TOK_CORE = TOK // N_CORES         # 32768 tokens per core
P = 128                           # partitions
C = 80                            # packed input columns per token
K_SEQ = [30, 45, 45, 46, 45, 45]  # tokens per partition per tile
GROUPS = [[0, 1, 2], [3, 4], [5]]  # tile groups for the small-op stage
NT = len(K_SEQ)
KMAX = max(K_SEQ)
assert P * sum(K_SEQ) == TOK_CORE
assert all(k * 32 * 32 < 2 ** 16 for k in K_SEQ)   # local_scatter dst limit
BASES = [P * sum(K_SEQ[:t]) for t in range(NT)]
M_G = [sum(K_SEQ[t] for t in g) for g in GROUPS]
GO = [P * sum(M_G[:i]) for i in range(len(GROUPS))]  # flg row offset per group
# which tiles compute the one-hot via V is_ge (rest: G subtract + S relu)
EQ_ON_V = {0}
# which tiles do the band add on V (rest on G)
BAND_ON_V = set()

F32 = mybir.dt.float32
BF16 = mybir.dt.bfloat16
FP16 = mybir.dt.float16
I32 = mybir.dt.int32
I16 = mybir.dt.int16
Op = mybir.AluOpType
Act = mybir.ActivationFunctionType

OFF8 = -(0.5 - 2.0 ** -9)         # floor bias, fraction grid 2^-8
OFF15 = -(0.5 - 2.0 ** -16)       # floor bias, fraction grid 2^-15
OFF19 = -(0.5 - 2.0 ** -20)       # floor bias, fraction grid 2^-19


def _build():
    nc = bacc.Bacc("TRN2", debug=False, enable_asserts=False, num_devices=N_CORES)
    x = nc.dram_tensor("x", [TOK_CORE, C], F32, kind="ExternalInput").ap()
    flg = nc.dram_tensor("flg", [TOK_CORE, 2], I16, kind="ExternalInput").ap()
    y = nc.dram_tensor("y", [TOK_CORE, 32], BF16, kind="ExternalOutput").ap()

    with tile.TileContext(nc) as tc, ExitStack() as ctx:
        io_pool = ctx.enter_context(tc.tile_pool(name="io", bufs=NT))
        eq_pool = ctx.enter_context(tc.tile_pool(name="eq", bufs=3))
        r3_pool = ctx.enter_context(tc.tile_pool(name="r3", bufs=3))
        pl_pool = ctx.enter_context(tc.tile_pool(name="pl", bufs=3))
        bd_pool = ctx.enter_context(tc.tile_pool(name="bd", bufs=3))
        gr_pool = ctx.enter_context(tc.tile_pool(name="gr", bufs=2))
        const_pool = ctx.enter_context(tc.tile_pool(name="const", bufs=1))

        # ---- all in-DMAs first on the Sync queue (bufs cover every tile,
        # so loads stream back-to-back from t~0, overlapping const setup).
        xts, flgts = [], []
        for t in range(NT):
            xt = io_pool.tile([P, K_SEQ[t] * C], F32, tag="xt")
            x_t = x[BASES[t]:BASES[t] + P * K_SEQ[t]].rearrange(
                "(p j) f -> p (j f)", p=P)
            nc.sync.dma_start(xt[:], x_t)
            xts.append(xt)
        for gi in range(len(GROUPS)):
            fm = gr_pool.tile([P, M_G[gi] * 2], I16, tag="flgt")
            f_t = flg[GO[gi]:GO[gi] + P * M_G[gi]].rearrange(
                "(p m) l -> p (m l)", p=P)
            nc.sync.dma_start(fm[:], f_t)
            flgts.append(fm)

        # ---- constants; local_scatter warmup first (~6us Q7 IRAM load) ----
        data2 = const_pool.tile([P, KMAX * 2], BF16)         # scatter payload
        nc.gpsimd.memset(data2[:], 2.0)
        wu_idx = const_pool.tile([P, 2], I16)
        nc.gpsimd.memset(wu_idx[:], -1)
        wu_dst = const_pool.tile([P, 4], BF16)
        nc.gpsimd.local_scatter(wu_dst[:], data2[:, 0:2], wu_idx[:],
                                channels=P, num_elems=4, num_idxs=2)

        # w48: [15-s | 240-16s | 2^(15-s)] per 16-bin group (all descending)
        w48 = const_pool.tile([P, 48], BF16)
        tmp_i = const_pool.tile([P, 32], I32)
        nc.gpsimd.iota(tmp_i[:], pattern=[[0, 2], [-1, 16]], base=15,
                       channel_multiplier=0)
        nc.scalar.copy(w48[:, 0:32], tmp_i[:])
        nc.vector.tensor_scalar(w48[:, 16:32], w48[:, 16:32], 16.0, None,
                                op0=Op.mult)
        tmp_h = const_pool.tile([P, 16], I16)               # bf16 bits of 2^(15-s)
        nc.gpsimd.iota(tmp_h[:], pattern=[[-128, 16]], base=(127 + 15) << 7,
                       channel_multiplier=0)
        nc.scalar.copy(w48[:, 32:48], tmp_h[:].bitcast(BF16))
        wrep = const_pool.tile([P, KMAX * 48], BF16)        # repeat per token
        nc.scalar.copy(wrep[:].rearrange("p (j f) -> p j f", j=KMAX),
                       w48[:].unsqueeze(1).broadcast_to([P, KMAX, 48]))

        # whole-core staging for the small-op stage
        rw_st = const_pool.tile([P, sum(K_SEQ), 3], BF16)

        def heavy(t):
            K = K_SEQ[t]
            x3 = xts[t][:].rearrange("p (j f) -> p j f", j=K)
            x48 = x3[:, :, 0:48].rearrange("p j (g s) -> p j g s", s=16)
            r3 = r3_pool.tile([P, K, 3], F32, tag="r3")
            nc.vector.tensor_reduce(r3[:], x48, axis=mybir.AxisListType.X,
                                    op=Op.max)
            equ = eq_pool.tile([P, K * 48], BF16, tag="equ")
            eq4 = equ[:].rearrange("p (j g s) -> p j g s", j=K, g=3)
            r3b = r3[:].unsqueeze(3).broadcast_to([P, K, 3, 16])
            if t in EQ_ON_V:
                nc.vector.tensor_tensor(eq4, x48, r3b, op=Op.is_ge)
            else:
                nc.gpsimd.tensor_tensor(eq4, x48, r3b, op=Op.subtract)
                nc.scalar.activation(equ[:], equ[:], Act.Relu, bias=1.0,
                                     scale=1e30)
            nc.vector.tensor_tensor(equ[:], equ[:], wrep[:, :K * 48],
                                    op=Op.mult)
            o = BASES[t] // P
            nc.vector.tensor_reduce(rw_st[:, o:o + K, :], eq4,
                                    axis=mybir.AxisListType.X, op=Op.max)

        def group_stage(gi):
            M = M_G[gi]
            o = BASES[GROUPS[gi][0]] // P
            rw = rw_st[:, o:o + M, :]
            flgv = flgts[gi][:].rearrange("p (m l) -> p m l", l=2)
            jmo0 = flgv.bitcast(FP16)[:, :, 0]

            t01 = gr_pool.tile([P, M], BF16, tag="t01")
            nc.vector.tensor_tensor(t01[:], rw[:, :, 0], rw[:, :, 1], op=Op.add)
            value = gr_pool.tile([P, M], BF16, tag="value")
            nc.scalar.activation(value[:], t01[:], Act.Copy, bias=255.0,
                                 scale=-1.0)
            p = gr_pool.tile([P, M], BF16, tag="p")         # 2^-shift
            nc.scalar.activation(p[:], rw[:, :, 2], Act.Copy, scale=2.0 ** -15)
            # 2^shift from bf16 bit identity: bits(2^sh) = 32512 - bits(2^-sh)
            pf = gr_pool.tile([P, M], I16, tag="pf")
            nc.scalar.activation(pf[:], p[:].bitcast(I16), Act.Copy,
                                 bias=32512.0, scale=-1.0)
            # p := shl ? 2^shift : 2^-shift   (shl wins over shr, as in ref)
            nc.vector.copy_predicated(p[:], flgv[:, :, 1], pf[:].bitcast(BF16))
            q = gr_pool.tile([P, M], BF16, tag="q")
            nc.vector.tensor_tensor(q[:], value[:], p[:], op=Op.mult)
            # t = q mod 256 via m = floor(q/256)
            m_i = gr_pool.tile([P, M], I32, tag="m_i")
            nc.scalar.activation(m_i[:], q[:], Act.Copy, bias=OFF8,
                                 scale=1.0 / 256.0)
            m_sc = gr_pool.tile([P, M], BF16, tag="m_sc")   # -256*m
            nc.scalar.activation(m_sc[:], m_i[:], Act.Copy, scale=-256.0)
            tq = gr_pool.tile([P, M], BF16, tag="tq")
            nc.vector.tensor_tensor(tq[:], q[:], m_sc[:], op=Op.add)
            # res = floor(t); hi = floor(t/16); lo = res - 16*hi
            res_i = gr_pool.tile([P, M], I32, tag="res_i")
            nc.scalar.activation(res_i[:], tq[:], Act.Copy, bias=OFF15)
            hi_i = gr_pool.tile([P, M], I32, tag="hi_i")
            nc.scalar.activation(hi_i[:], tq[:], Act.Copy, bias=OFF19,
                                 scale=1.0 / 16.0)
            res_f = gr_pool.tile([P, M], BF16, tag="res_f")
            nc.scalar.copy(res_f[:], res_i[:])
            hi16 = gr_pool.tile([P, M], BF16, tag="hi16")   # 16*hi
            nc.scalar.activation(hi16[:], hi_i[:], Act.Copy, scale=16.0)
            pair = gr_pool.tile([P, M, 2], FP16, tag="pair")
            nc.scalar.activation(pair[:, :, 1], hi_i[:], Act.Copy, bias=16.0)
            nc.vector.tensor_tensor(pair[:, :, 0], res_f[:], hi16[:],
                                    op=Op.subtract)
            idxf = gr_pool.tile([P, M, 2], FP16, tag="idxf")
            jmo0b = jmo0.unsqueeze(2).broadcast_to([P, M, 2])
            nc.gpsimd.tensor_tensor(idxf[:], pair[:], jmo0b, op=Op.add)
            idx16 = gr_pool.tile([P, M * 2], I16, tag="idx16")
            nc.scalar.copy(idx16[:], idxf[:].rearrange("p j l -> p (j l)"))
            return idx16

        def tail(t, idx16, ko):
            K = K_SEQ[t]
            x3 = xts[t][:].rearrange("p (j f) -> p j f", j=K)
            plane = pl_pool.tile([P, K * 32], BF16, tag="plane")
            nc.gpsimd.local_scatter(
                plane[:], data2[:, 0:K * 2], idx16[:, ko * 2:(ko + K) * 2],
                channels=P, num_elems=K * 32, num_idxs=K * 2)
            band = bd_pool.tile([P, K * 32], BF16, tag="band")
            eng = nc.vector if t in BAND_ON_V else nc.gpsimd
            eng.tensor_tensor(
                band[:].rearrange("p (j s) -> p j s", j=K),
                x3[:, :, 48:80],
                plane[:].rearrange("p (j s) -> p j s", j=K),
                op=Op.add)
            y_t = y[BASES[t]:BASES[t] + P * K].rearrange(
                "(p j) f -> p (j f)", p=P)
            nc.scalar.dma_start(y_t, band[:])

        for gi, g in enumerate(GROUPS):
            for t in g:
                heavy(t)
            idx16 = group_stage(gi)
            ko = 0
            for t in g:
                tail(t, idx16, ko)
                ko += K_SEQ[t]

    nc.compile()
    return nc


_NC_CACHE = None
_HOST_CACHE = None


def _get_nc():
    global _NC_CACHE
    if _NC_CACHE is None:
        _NC_CACHE = _build()
    return _NC_CACHE


def _host_maps():
    """Per-core row gather map for flg (group-staged order) + 32*j values."""
    global _HOST_CACHE
    if _HOST_CACHE is None:
        rows, jvs = [], []
        for g in GROUPS:
            pr = np.arange(P)[:, None]
            r = np.concatenate(
                [BASES[t] + pr * K_SEQ[t] + np.arange(K_SEQ[t])[None, :]
                 for t in g], axis=1)                      # [P, M_g]
            jv = np.concatenate(
                [np.broadcast_to(32 * np.arange(K_SEQ[t], dtype=np.int32),
                                 (P, K_SEQ[t])) for t in g], axis=1)
            rows.append(r.reshape(-1))
            jvs.append(jv.reshape(-1))
        _HOST_CACHE = (np.concatenate(rows), np.concatenate(jvs))
    return _HOST_CACHE


def kernel(x_bd: np.ndarray, _trace: bool = False, **_kw):
    assert x_bd.shape == (B, S, D) and x_bd.dtype == np.float32
    nc = _get_nc()
    xf = np.ascontiguousarray(x_bd).reshape(TOK, D)
    x80 = np.empty((TOK, C), np.float32)
    x80[:, 0:48] = xf[:, 16:64]
    x80[:, 48:80] = xf[:, 64:96]

    # flg: (jmo0 = 32*j - 8192*(1-active) as fp16 bits, shl) in staged order
    rows, jv = _host_maps()
    mark = xf[:, 0] >= 0.5
    shl = xf[:, 1] > 0.5
    shr = xf[:, 2] > 0.5
    off = np.where(mark & (shl | shr), np.float32(0), np.float32(8192))
    flg_all = np.empty((TOK, 2), np.int16)
    in_maps = []
    for c in range(N_CORES):
        cb = c * TOK_CORE
        jmo0 = (jv.astype(np.float32) - off[cb + rows]).astype(np.float16)
        fc = flg_all[cb:cb + TOK_CORE]
        fc[:, 0] = jmo0.view(np.int16)
        fc[:, 1] = shl[cb + rows].astype(np.int16)
        in_maps.append({"x": x80[cb:cb + TOK_CORE], "flg": fc})
    res = run_bass_kernel_spmd(nc, in_maps, core_ids=list(range(N_CORES)),
                               trace=_trace)
    band = np.concatenate([np.asarray(res.results[c]["y"])
                           for c in range(N_CORES)], axis=0)
    out = np.array(xf, copy=True)
    out[:, 64:96] = band.astype(np.float32)
    out = out.reshape(B, S, D)
    if _trace:
        return out, res
    return out


# revision 13
# speedup vs baseline: 1.7730x; 1.6109x over previous
"""Trainium2 Bass kernel for nn_ByteShiftPowerOf2 (v5).

Reference semantics per token (B*S tokens, D=128 features):
  val_lo = argmax(x[16:32]); val_hi = argmax(x[32:48]); value = val_lo + 16*val_hi
  shift  = argmax(x[48:64])
  mark = x[0] >= 0.5; shl = x[1] > 0.5; shr = x[2] > 0.5; active = mark & (shl|shr)
  result = shl ? (value << shift) & 255 : value >> shift
  out = x; if active: out[64 + (result & 15)] += 2.0; out[80 + (result >> 4)] += 2.0

Split of work: the device runs the three 16-bin argmaxes, the byte-shift
arithmetic, and builds the +2.0 scatter plane per token (the scatter_memory
pattern); the host routes data: packs the 48 argmax columns (192B/token)
plus a 6B/token flag sidecar in, and adds the returned bf16 plane (exact
values {0,2}) onto x[64:96] in f32 - bit-identical to the reference.
HBM/core: 7.9 MB in + 2.1 MB out.

Device pipeline notes:
 - int32 DVE ops measure ~20x slower than f32 -> all-float index math.
 - mod/divide don't exist on TRN2; floor() is done via the round-to-
   nearest f32->i32 convert biased by -(0.5 - grid/2): all values sit on
   known power-of-2 grids, so the RNE convert is exactly floor.
 - gpsimd swaps its Q7 IRAM library (~6us) between tensor ops and
   local_scatter, so gpsimd runs all subtracts first, then only scatters.
 - eq one-hot: d = x48 - max (gpsimd, V is_ge for tile 0 for balance),
   Relu(d*1e30+1) on ACT; u = eq*w with w = [15-s | 240-16s | 2^(15-s)]
   (descending => first-occurrence tie-break like jnp.argmax);
   reduce_max(u) = (15-lo, 240-16hi, 2^(15-shift)).
 - 2^shift from the bf16 bit identity bits(2^sh) = 32512 - bits(2^-sh).
 - q = value * (shl ? 2^sh : 2^-sh) is bf16-exact (8-bit significands,
   power-of-two scales); t = q mod 256 via floor; nibbles via floor.
 - idx = (lo, hi+16) + (32*j - 8192*(1-active)) with the j/deactivation
   part host-precomputed (fp16 sidecar); inactive tokens go negative and
   local_scatter skips them.

6 tiles (K: 30,45,45,46,45,45) pipeline the DMA; the per-token scalar
chain runs once per tile group (120/91/45 tokens) to amortize per-
instruction fixed costs, groups shrinking toward the end for a short
drain tail. Emission is interleaved per group so every engine's in-order
queue follows pipeline order.
"""

import numpy as np
from contextlib import ExitStack

import concourse.bass as bass
import concourse.tile as tile
from concourse import bacc, mybir
from concourse.bass_utils import run_bass_kernel_spmd

B, S, D = 32, 8192, 128
N_CORES = 8
TOK = B * S                       # 262144 tokens
TOK_CORE = TOK // N_CORES         # 32768 tokens per core
P = 128                           # partitions
C = 48                            # packed input columns per token
K_SEQ = [30, 45, 45, 46, 45, 45]  # tokens per partition per tile
GROUPS = [[0, 1, 2], [3, 4], [5]]  # tile groups for the small-op stage
NT = len(K_SEQ)
KMAX = max(K_SEQ)
assert P * sum(K_SEQ) == TOK_CORE
assert all(k * 32 * 32 < 2 ** 16 for k in K_SEQ)   # local_scatter dst limit
BASES = [P * sum(K_SEQ[:t]) for t in range(NT)]
M_G = [sum(K_SEQ[t] for t in g) for g in GROUPS]
GO = [P * sum(M_G[:i]) for i in range(len(GROUPS))]  # flg row offset per group
# tiles whose one-hot compare runs on Vector (is_ge); rest on GpSimd+ACT.
# Keep empty: gpsimd must run all its tensor ops before all local_scatters,
# since each tensor<->scatter transition swaps the Q7 IRAM library (~6us).
EQ_ON_V = set()

F32 = mybir.dt.float32
BF16 = mybir.dt.bfloat16
FP16 = mybir.dt.float16
I32 = mybir.dt.int32
I16 = mybir.dt.int16
Op = mybir.AluOpType
Act = mybir.ActivationFunctionType

OFF8 = -(0.5 - 2.0 ** -9)         # floor bias, fraction grid 2^-8
OFF15 = -(0.5 - 2.0 ** -16)       # floor bias, fraction grid 2^-15
OFF19 = -(0.5 - 2.0 ** -20)       # floor bias, fraction grid 2^-19


def _build():
    nc = bacc.Bacc("TRN2", debug=False, enable_asserts=False, num_devices=N_CORES)
    x = nc.dram_tensor("x", [TOK_CORE, C], F32, kind="ExternalInput").ap()
    flg = nc.dram_tensor("flg", [TOK_CORE, 3], I16, kind="ExternalInput").ap()
    y = nc.dram_tensor("y", [TOK_CORE, 32], BF16, kind="ExternalOutput").ap()

    with tile.TileContext(nc) as tc, ExitStack() as ctx:
        io_pool = ctx.enter_context(tc.tile_pool(name="io", bufs=NT))
        fl_pool = ctx.enter_context(tc.tile_pool(name="fl", bufs=3))
        eq_pool = ctx.enter_context(tc.tile_pool(name="eq", bufs=3))
        r3_pool = ctx.enter_context(tc.tile_pool(name="r3", bufs=3))
        pl_pool = ctx.enter_context(tc.tile_pool(name="pl", bufs=3))
        gr_pool = ctx.enter_context(tc.tile_pool(name="gr", bufs=2))
        const_pool = ctx.enter_context(tc.tile_pool(name="const", bufs=1))

        # ---- all in-DMAs first on the Sync queue (bufs cover every tile,
        # so loads stream back-to-back from t~0, overlapping const setup).
        xts, flgts = [], []
        for t in range(NT):
            xt = io_pool.tile([P, K_SEQ[t] * C], F32, tag="xt")
            x_t = x[BASES[t]:BASES[t] + P * K_SEQ[t]].rearrange(
                "(p j) f -> p (j f)", p=P)
            nc.sync.dma_start(xt[:], x_t)
            xts.append(xt)
        for gi in range(len(GROUPS)):
            fm = fl_pool.tile([P, M_G[gi] * 3], I16, tag="flgt")
            f_t = flg[GO[gi]:GO[gi] + P * M_G[gi]].rearrange(
                "(p m) l -> p (m l)", p=P)
            nc.sync.dma_start(fm[:], f_t)
            flgts.append(fm)

        # ---- constants; local_scatter warmup first (~6us Q7 IRAM load) ----
        data2 = const_pool.tile([P, KMAX * 2], BF16)         # scatter payload
        nc.gpsimd.memset(data2[:], 2.0)
        wu_idx = const_pool.tile([P, 2], I16)
        nc.gpsimd.memset(wu_idx[:], -1)
        wu_dst = const_pool.tile([P, 4], BF16)
        nc.gpsimd.local_scatter(wu_dst[:], data2[:, 0:2], wu_idx[:],
                                channels=P, num_elems=4, num_idxs=2)

        # w48: [15-s | 240-16s | 2^(15-s)] per 16-bin group (all descending)
        w48 = const_pool.tile([P, 48], BF16)
        tmp_i = const_pool.tile([P, 32], I32)
        nc.gpsimd.iota(tmp_i[:], pattern=[[0, 2], [-1, 16]], base=15,
                       channel_multiplier=0)
        nc.scalar.copy(w48[:, 0:32], tmp_i[:])
        nc.vector.tensor_scalar(w48[:, 16:32], w48[:, 16:32], 16.0, None,
                                op0=Op.mult)
        tmp_h = const_pool.tile([P, 16], I16)               # bf16 bits of 2^(15-s)
        nc.gpsimd.iota(tmp_h[:], pattern=[[-128, 16]], base=(127 + 15) << 7,
                       channel_multiplier=0)
        nc.scalar.copy(w48[:, 32:48], tmp_h[:].bitcast(BF16))
        wrep = const_pool.tile([P, KMAX * 48], BF16)        # repeat per token
        nc.scalar.copy(wrep[:].rearrange("p (j f) -> p j f", j=KMAX),
                       w48[:].unsqueeze(1).broadcast_to([P, KMAX, 48]))

        # whole-core staging for the small-op stage
        rw_st = const_pool.tile([P, sum(K_SEQ), 3], BF16)

        def heavy(t):
            K = K_SEQ[t]
            x48 = xts[t][:].rearrange("p (j g s) -> p j g s", j=K, g=3)
            r3 = r3_pool.tile([P, K, 3], F32, tag="r3")
            nc.vector.tensor_reduce(r3[:], x48, axis=mybir.AxisListType.X,
                                    op=Op.max)
            equ = eq_pool.tile([P, K * 48], BF16, tag="equ")
            eq4 = equ[:].rearrange("p (j g s) -> p j g s", j=K, g=3)
            r3b = r3[:].unsqueeze(3).broadcast_to([P, K, 3, 16])
            if t in EQ_ON_V:
                nc.vector.tensor_tensor(eq4, x48, r3b, op=Op.is_ge)
            else:
                nc.gpsimd.tensor_tensor(eq4, x48, r3b, op=Op.subtract)
                nc.scalar.activation(equ[:], equ[:], Act.Relu, bias=1.0,
                                     scale=1e30)
            nc.vector.tensor_tensor(equ[:], equ[:], wrep[:, :K * 48],
                                    op=Op.mult)
            o = BASES[t] // P
            nc.vector.tensor_reduce(rw_st[:, o:o + K, :], eq4,
                                    axis=mybir.AxisListType.X, op=Op.max)

        def group_stage(gi):
            M = M_G[gi]
            o = BASES[GROUPS[gi][0]] // P
            rw = rw_st[:, o:o + M, :]
            flgv = flgts[gi][:].rearrange("p (m l) -> p m l", l=3)
            jmo = flgv[:, :, 0:2].bitcast(FP16)

            t01 = gr_pool.tile([P, M], BF16, tag="t01")
            nc.vector.tensor_tensor(t01[:], rw[:, :, 0], rw[:, :, 1], op=Op.add)
            value = gr_pool.tile([P, M], BF16, tag="value")
            nc.scalar.activation(value[:], t01[:], Act.Copy, bias=255.0,
                                 scale=-1.0)
            p = gr_pool.tile([P, M], BF16, tag="p")         # 2^-shift
            nc.scalar.activation(p[:], rw[:, :, 2], Act.Copy, scale=2.0 ** -15)
            # 2^shift from bf16 bit identity: bits(2^sh) = 32512 - bits(2^-sh)
            pf = gr_pool.tile([P, M], I16, tag="pf")
            nc.scalar.activation(pf[:], p[:].bitcast(I16), Act.Copy,
                                 bias=32512.0, scale=-1.0)
            # p := shl ? 2^shift : 2^-shift   (shl wins over shr, as in ref)
            nc.vector.copy_predicated(p[:], flgv[:, :, 2], pf[:].bitcast(BF16))
            q = gr_pool.tile([P, M], BF16, tag="q")
            nc.vector.tensor_tensor(q[:], value[:], p[:], op=Op.mult)
            # t = q mod 256 via m = floor(q/256)
            m_i = gr_pool.tile([P, M], I32, tag="m_i")
            nc.scalar.activation(m_i[:], q[:], Act.Copy, bias=OFF8,
                                 scale=1.0 / 256.0)
            m_sc = gr_pool.tile([P, M], BF16, tag="m_sc")   # -256*m
            nc.scalar.activation(m_sc[:], m_i[:], Act.Copy, scale=-256.0)
            tq = gr_pool.tile([P, M], BF16, tag="tq")
            nc.vector.tensor_tensor(tq[:], q[:], m_sc[:], op=Op.add)
            # res = floor(t); hi = floor(t/16); lo = res - 16*hi
            res_i = gr_pool.tile([P, M], I32, tag="res_i")
            nc.scalar.activation(res_i[:], tq[:], Act.Copy, bias=OFF15)
            hi_i = gr_pool.tile([P, M], I32, tag="hi_i")
            nc.scalar.activation(hi_i[:], tq[:], Act.Copy, bias=OFF19,
                                 scale=1.0 / 16.0)
            res_f = gr_pool.tile([P, M], BF16, tag="res_f")
            nc.scalar.copy(res_f[:], res_i[:])
            hi16 = gr_pool.tile([P, M], BF16, tag="hi16")   # 16*hi
            nc.scalar.activation(hi16[:], hi_i[:], Act.Copy, scale=16.0)
            pair = gr_pool.tile([P, M, 2], FP16, tag="pair")
            nc.scalar.copy(pair[:, :, 1], hi_i[:])   # +16 comes via jmo lane 1
            nc.vector.tensor_tensor(pair[:, :, 0], res_f[:], hi16[:],
                                    op=Op.subtract)
            idxf = gr_pool.tile([P, M, 2], FP16, tag="idxf")
            nc.vector.tensor_tensor(idxf[:], pair[:], jmo, op=Op.add)
            idx16 = gr_pool.tile([P, M * 2], I16, tag="idx16")
            nc.scalar.copy(idx16[:], idxf[:].rearrange("p j l -> p (j l)"))
            return idx16

        def tail(t, idx16, ko):
            K = K_SEQ[t]
            plane = pl_pool.tile([P, K * 32], BF16, tag="plane")
            nc.gpsimd.local_scatter(
                plane[:], data2[:, 0:K * 2], idx16[:, ko * 2:(ko + K) * 2],
                channels=P, num_elems=K * 32, num_idxs=K * 2)
            y_t = y[BASES[t]:BASES[t] + P * K].rearrange(
                "(p j) f -> p (j f)", p=P)
            nc.scalar.dma_start(y_t, plane[:])

        # Phase A: all heavy passes; B: all group stages; C: all scatters.
        # This keeps gpsimd's in-order queue as [subs... | lib swap |
        # scatters...] with a single library transition.
        for t in range(NT):
            heavy(t)
        idx16s = [group_stage(gi) for gi in range(len(GROUPS))]
        for gi, g in enumerate(GROUPS):
            ko = 0
            for t in g:
                tail(t, idx16s[gi], ko)
                ko += K_SEQ[t]

    nc.compile()
    return nc


_NC_CACHE = None
_HOST_CACHE = None


def _get_nc():
    global _NC_CACHE
    if _NC_CACHE is None:
        _NC_CACHE = _build()
    return _NC_CACHE


def _host_maps():
    """Per-core row gather map for flg (group-staged order) + 32*j values."""
    global _HOST_CACHE
    if _HOST_CACHE is None:
        rows, jvs = [], []
        for g in GROUPS:
            pr = np.arange(P)[:, None]
            r = np.concatenate(
                [BASES[t] + pr * K_SEQ[t] + np.arange(K_SEQ[t])[None, :]
                 for t in g], axis=1)                      # [P, M_g]
            jv = np.concatenate(
                [np.broadcast_to(32 * np.arange(K_SEQ[t], dtype=np.int32),
                                 (P, K_SEQ[t])) for t in g], axis=1)
            rows.append(r.reshape(-1))
            jvs.append(jv.reshape(-1))
        _HOST_CACHE = (np.concatenate(rows), np.concatenate(jvs))
    return _HOST_CACHE


def kernel(x_bd: np.ndarray, _trace: bool = False, **_kw):
    assert x_bd.shape == (B, S, D) and x_bd.dtype == np.float32
    nc = _get_nc()
    xf = np.ascontiguousarray(x_bd).reshape(TOK, D)
    x48 = np.ascontiguousarray(xf[:, 16:64])

    # flg: (jmo0, jmo0+16 as fp16 bits, shl) in group-staged order, where
    # jmo0 = 32*j - 8192*(1-active)
    rows, jv = _host_maps()
    mark = xf[:, 0] >= 0.5
    shl = xf[:, 1] > 0.5
    shr = xf[:, 2] > 0.5
    off = np.where(mark & (shl | shr), np.float32(0), np.float32(8192))
    flg_all = np.empty((TOK, 3), np.int16)
    in_maps = []
    for c in range(N_CORES):
        cb = c * TOK_CORE
        base = jv.astype(np.float32) - off[cb + rows]
        fc = flg_all[cb:cb + TOK_CORE]
        fc[:, 0] = base.astype(np.float16).view(np.int16)
        fc[:, 1] = (base + 16.0).astype(np.float16).view(np.int16)
        fc[:, 2] = shl[cb + rows].astype(np.int16)
        in_maps.append({"x": x48[cb:cb + TOK_CORE], "flg": fc})
    res = run_bass_kernel_spmd(nc, in_maps, core_ids=list(range(N_CORES)),
                               trace=_trace)
    plane = np.concatenate([np.asarray(res.results[c]["y"])
                            for c in range(N_CORES)], axis=0)
    out = np.array(xf, copy=True)
    out[:, 64:96] += plane.astype(np.float32)
    out = out.reshape(B, S, D)
    if _trace:
        return out, res
    return out
